# revision 4
# baseline (speedup 1.0000x reference)
"""Trainium2 Bass kernel v2 for the 3D AttentionBlock.

Same SPMD decomposition as v1 (core = batch x spatial-quarter, host-side roll
makes programs uniform), with a restructured device program:

- x ships as bf16 only (no fp8 + conversion); the query quarter is a slice.
- GroupNorm rsqrt via DVE Quake bit-trick + 2 Newton steps: the ACT engine
  only ever runs the exp_and_others table -> zero table loads in steady state.
- K bias dropped entirely (a per-channel bias on K adds a per-query constant
  to every key score, which softmax cancels); V bias folded into the proj
  bias via pb' = pb + Wp @ (Wv t + bv) using the denominator identity
  (V+b)P = VP + b (1^T P); Q bias kept on the ACT pass.
- V^T computed directly per m-tile as xps_mt^T @ Wv (contraction over
  channels on partitions) -- no [C,M] V buffer, no PE transposes.
- Pooling first-level adds run on GPSIMD (SBUF-only engine, otherwise idle);
  second/third level on DVE.
- The attention inner loop is software-pipelined on the PE stream: AV for
  group g is emitted after scores for group g+1, so the in-order PE queue
  never blocks on exp(g) while scores(g+1) is ready. exp alternates
  ACT (real Exp) / DVE (Schraudolph int16 affine) per a tunable pattern.
- The denominator broadcast uses a single 4-partition selector matmul
  (sel4 [4, C]) instead of four K=1 matmuls.
- Output is stored per 512-block as soon as its boundary completes.
"""

import numpy as np
import ml_dtypes
from contextlib import ExitStack

import concourse.bass as bass
import concourse.tile as tile
from concourse import mybir
from concourse.bacc import Bacc
from concourse.bass_utils import run_bass_kernel_spmd

F32 = mybir.dt.float32
F32R = mybir.dt.float32r
BF16 = mybir.dt.bfloat16
I16 = mybir.dt.int16
I32 = mybir.dt.int32
AF = mybir.ActivationFunctionType
ALU = mybir.AluOpType

C = 128
SP = 13824
NQ = SP // 4
M = 1728
MP = 1792
NMT = MP // 128
NH = 4
HD = 32
EPS = 1e-5
BLOCKS = [512] * 6 + [384]
NGRP = 28

# packed f32 const layout: [0:384] wq (q-scaled), [384:387] bq, [387] pb,
# [388] gnw, [389] gnb, [390:398] gsum, [398] quake magic bits
CPK_W = 399
QUAKE = np.frombuffer(np.uint32(0x5F3759DF).tobytes(), np.float32)[0]

A16 = 128.0 / float(np.log(2.0))
B16 = 127.0 * 128.0

_CACHE = {}

CFG = dict(
    epat="ADADADADADA",  # exp engine per group: A=ACT exp, D=DVE Schraudolph
    av_lag=2,       # groups between scores(g) and AV(g) on the PE stream
    bnd1_lag=2,     # groups into block b before s4 copies of block b-1
    bnd2_lag=5,     # groups into block b before denom/normalize of b-1
    bnd3_lag=9,     # groups into block b before proj/residual/store of b-1
    stats_frac=2,   # GroupNorm stats sample every Nth 432-col chunk
    ablate=None,    # None | 'no_exp' | 'no_av' | 'depack' (timing probes)
)


def _load_consts(nc, ctx, tc, cpk, cpb, gbr, sel4):
    pool = ctx.enter_context(tc.tile_pool(name="const", bufs=1))
    dma = nc.default_dma_engine
    cpk_t = pool.tile([C, CPK_W], F32, tag="cpk")
    dma.dma_start(out=cpk_t, in_=cpk[:, :])
    cpb_t = pool.tile([C, C], BF16, tag="cpb")
    dma.dma_start(out=cpb_t, in_=cpb[:, :])
    gbr_t = pool.tile([8, C], F32R, tag="gbr")
    dma.dma_start(out=gbr_t, in_=gbr[:, :])
    sel4_t = pool.tile([C, C], F32R, tag="sel4")
    dma.dma_start(out=sel4_t, in_=sel4[:, :])
    return dict(
        wq=cpk_t[:, 0:3 * C], bq=cpk_t[:, 3 * C:3 * C + 3],
        pb=cpk_t[:, 387:388], gnw=cpk_t[:, 388:389], gnb=cpk_t[:, 389:390],
        gsum=cpk_t[:, 390:398], quake=cpk_t[:, 398:399],
        wp=cpb_t, gbr=gbr_t, sel4=sel4_t)


def _body(nc, ctx, tc, ct, x, out):
    sb = ctx.enter_context(tc.tile_pool(name="sb", bufs=1))
    work = ctx.enter_context(tc.tile_pool(name="work", bufs=2))
    ptp = ctx.enter_context(tc.tile_pool(name="ptp", bufs=6))
    stg = ctx.enter_context(tc.tile_pool(name="stg", bufs=3))
    ps = ctx.enter_context(tc.tile_pool(name="ps", bufs=1, space="PSUM"))

    dma = nc.default_dma_engine   # SP HWDGE
    adma = nc.scalar              # ACT HWDGE

    # ---------------- load x + stats (chunked so stats chase the DMA) -----
    # GroupNorm stats sample every stats_frac'th 432-col chunk: the group
    # variance averages over >=110k samples either way (sampling error
    # ~0.2%, far inside the error budget) and it halves the DVE front.
    sfrac = CFG["stats_frac"]
    NST = 32 // sfrac
    x_sb = sb.tile([C, SP], BF16, tag="x")
    stats = sb.tile([C, NST, 6], F32, tag="stats")
    HALF = SP // 2
    CH = 1728
    si = 0
    for ci in range(8):
        eng = dma if ci % 2 == 0 else adma
        lo = (ci % 2) * HALF + (ci // 2) * CH
        eng.dma_start(out=x_sb[:, lo:lo + CH], in_=x[:, lo:lo + CH])
        for j in range(4):
            if (4 * ci + j) % sfrac == 0:
                nc.vector.bn_stats(out=stats[:, si, :],
                                   in_=x_sb[:, lo + j * 432:lo + (j + 1) * 432])
                si += 1

    # ------------- GroupNorm stats -> per-channel scale/shift -------------
    mv = sb.tile([C, 2], F32, tag="mv")
    nc.vector.bn_aggr(out=mv, in_=stats)
    m12 = sb.tile([C, 2], F32R, tag="m12")          # [mean_c, E[x^2]_c]
    nc.vector.tensor_copy(out=m12[:, 0:1], in_=mv[:, 0:1])
    nc.vector.tensor_tensor(out=m12[:, 1:2], in0=mv[:, 0:1], in1=mv[:, 0:1],
                            op=ALU.mult)
    nc.vector.tensor_tensor(out=m12[:, 1:2], in0=m12[:, 1:2], in1=mv[:, 1:2],
                            op=ALU.add)

    # pooled sums (8x the pooled mean), bf16; first-level adds on GPSIMD
    # (emitted for all 4 steps up front so the GPSIMD chain starts early),
    # second/third level on DVE, deferred per-step until needed.
    xps = sb.tile([C, M], BF16, tag="xps")
    t1s = []
    for st in range(4):
        base = st * 3456
        xv = x_sb[:, base:base + 3456].rearrange(
            "p (h w d t) -> p h w d t", h=6, w=24, d=12, t=2)
        t1 = work.tile([C, 6, 24, 12], BF16, tag="t1", bufs=4)
        nc.gpsimd.tensor_tensor(out=t1, in0=xv[:, :, :, :, 0],
                                in1=xv[:, :, :, :, 1], op=ALU.add)
        t1s.append(t1)

    def t23_step(st):
        t1v = t1s[st].rearrange("p h (w t) d -> p h w t d", t=2)
        t2 = work.tile([C, 6, 12, 12], BF16, tag="t2")
        nc.vector.tensor_tensor(out=t2, in0=t1v[:, :, :, 0, :],
                                in1=t1v[:, :, :, 1, :], op=ALU.add)
        t2v = t2.rearrange("p (h t) w d -> p h t w d", t=2)
        ov = xps[:, st * 432:(st + 1) * 432].rearrange(
            "p (h w d) -> p h w d", h=3, w=12)
        nc.vector.tensor_tensor(out=ov, in0=t2v[:, :, 0, :, :],
                                in1=t2v[:, :, 1, :, :], op=ALU.add)

    g_ps = ps.tile([8, 2], F32, tag="av", bufs=4)
    nc.tensor.matmul(g_ps, ct["gsum"], m12.bitcast(F32), start=True, stop=True)
    g_sb = sb.tile([8, 2], F32R, tag="gsb")
    nc.vector.tensor_copy(out=g_sb, in_=g_ps)
    bc_ps = ps.tile([C, 2], F32, tag="av", bufs=4)
    nc.tensor.matmul(bc_ps, ct["gbr"].bitcast(F32), g_sb.bitcast(F32),
                     start=True, stop=True)
    bc = sb.tile([C, 2], F32, tag="bc")     # [mu_g, E_g[x^2]] per channel
    nc.vector.tensor_copy(out=bc, in_=bc_ps)
    var_t = sb.tile([C, 1], F32, tag="var")
    nc.vector.tensor_tensor(out=var_t, in0=bc[:, 0:1], in1=bc[:, 0:1],
                            op=ALU.mult)
    nc.vector.tensor_tensor(out=var_t, in0=bc[:, 1:2], in1=var_t,
                            op=ALU.subtract)
    nc.vector.tensor_scalar_add(out=var_t, in0=var_t, scalar1=EPS)
    # preload the exp table while ACT is idle (only table load ever needed)
    warm_t = sb.tile([C, 1], F32, tag="warm")
    nc.scalar.activation(out=warm_t, in_=var_t, func=AF.Exp)
    # Quake rsqrt + 2 Newton steps: r_t = 1/sqrt(var+eps)
    sh_t = sb.tile([C, 1], I32, tag="sh")
    nc.vector.tensor_scalar(out=sh_t, in0=var_t.bitcast(I32),
                            scalar1=1, scalar2=None,
                            op0=ALU.logical_shift_right)
    r_t = sb.tile([C, 1], F32, tag="rt")
    nc.vector.tensor_tensor(out=r_t.bitcast(I32),
                            in0=ct["quake"].bitcast(I32), in1=sh_t,
                            op=ALU.subtract)
    nt_t = sb.tile([C, 1], F32, tag="nt")
    for _ in range(2):
        nc.vector.tensor_tensor(out=nt_t, in0=var_t, in1=r_t, op=ALU.mult)
        nc.vector.tensor_tensor(out=nt_t, in0=nt_t, in1=r_t, op=ALU.mult)
        nc.vector.tensor_scalar(out=nt_t, in0=nt_t, scalar1=-0.5, scalar2=1.5,
                                op0=ALU.mult, op1=ALU.add)
        nc.vector.tensor_tensor(out=r_t, in0=r_t, in1=nt_t, op=ALU.mult)
    s_t = sb.tile([C, 1], F32, tag="st")     # s_c = gamma_c * rsqrt(var+eps)
    nc.vector.tensor_tensor(out=s_t, in0=r_t, in1=ct["gnw"], op=ALU.mult)
    s8_t = sb.tile([C, 1], F32, tag="s8")    # s_c / 8 (pool mean fold)
    nc.vector.tensor_scalar_mul(out=s8_t, in0=s_t, scalar1=0.125)
    tt_t = sb.tile([C, 1], F32R, tag="tt")   # t_c = beta_c - mu_c * s_c
    nc.vector.tensor_tensor(out=tt_t, in0=bc[:, 0:1], in1=s_t, op=ALU.mult)
    nc.vector.tensor_tensor(out=tt_t, in0=ct["gnb"], in1=tt_t, op=ALU.subtract)

    # ---------------- fold GN into QKV weights / biases ----------------
    wsc = sb.tile([C, 3 * C], BF16, tag="wsc")
    nc.vector.tensor_scalar_mul(out=wsc[:, 0:C], in0=ct["wq"][:, 0:C],
                                scalar1=s_t)
    nc.vector.tensor_scalar_mul(out=wsc[:, C:3 * C], in0=ct["wq"][:, C:3 * C],
                                scalar1=s8_t)
    b_ps = ps.tile([C, 3], F32, tag="av", bufs=4)
    for j in (0, 2):
        nc.tensor.matmul(b_ps[:, j:j + 1], ct["wq"][:, j * C:(j + 1) * C],
                         tt_t.bitcast(F32), start=True, stop=True)
    b_sb = sb.tile([C, 3], F32, tag="bsb")
    nc.vector.tensor_tensor(out=b_sb, in0=b_ps, in1=ct["bq"], op=ALU.add)
    # pb2 = pb + Wp @ (Wv t + bv)  (V bias folded through attention+proj)
    bv16 = sb.tile([C, 1], BF16, tag="bv16")
    nc.vector.tensor_copy(out=bv16, in_=b_sb[:, 2:3])
    pb_ps = ps.tile([C, 1], F32, tag="av", bufs=4)
    nc.tensor.matmul(pb_ps, ct["wp"], bv16, start=True, stop=True)
    pb2 = sb.tile([C, 1], F32R, tag="pb2")
    nc.vector.tensor_tensor(out=pb2, in0=pb_ps, in1=ct["pb"], op=ALU.add)

    # ---------------- K (no bias), Q, V^T step emitters ----------------
    k_sb = sb.tile([C, MP], BF16, tag="ksb")
    nc.gpsimd.memset(k_sb[:, M:MP], 0.0)

    def k_step(j):
        lo = j * 432
        k_ps = ps.tile([C, 512], F32, tag="s3", bufs=2)
        nc.tensor.matmul(k_ps[:, 0:432], wsc[:, C:2 * C],
                         xps[:, lo:lo + 432], start=True, stop=True)
        nc.scalar.activation(out=k_sb[:, lo:lo + 432], in_=k_ps[:, 0:432],
                             func=AF.Copy)

    q_sb = sb.tile([C, NQ], BF16, tag="qsb")
    qoff = [0]
    for w in BLOCKS[:-1]:
        qoff.append(qoff[-1] + w)

    def q_step(b):
        w, off = BLOCKS[b], qoff[b]
        q_ps = ps.tile([C, 512], F32, tag="s3", bufs=2)
        nc.tensor.matmul(q_ps[:, 0:w], wsc[:, 0:C],
                         x_sb[:, off:off + w], start=True, stop=True)
        nc.scalar.activation(out=q_sb[:, off:off + w], in_=q_ps[:, 0:w],
                             func=AF.Identity, bias=b_sb[:, 0:1])

    # vTa[key, mt, h, 0:32] = V^T for head h of m-tile mt; col 32 = 1.0 so
    # the AV matmul also emits the softmax denominator as a 33rd output row.
    vTa = sb.tile([C, NMT, NH, 33], BF16, tag="vta")
    nc.vector.memset(vTa[:, :, :, 32:33], 1.0)
    nc.vector.memset(vTa[64:128, NMT - 1, :, 0:32], 0.0)

    def vt_step(mt):
        mw = 128 if mt < NMT - 1 else M - 128 * (NMT - 1)
        vt_ps = ps.tile([C, C], F32, tag="av", bufs=4)
        nc.tensor.matmul(vt_ps[0:mw, :], xps[:, mt * 128:mt * 128 + mw],
                         wsc[:, 2 * C:3 * C], start=True, stop=True)
        nc.scalar.activation(
            out=vTa[0:mw, mt, :, 0:32],
            in_=vt_ps[0:mw, :].rearrange("p (h d) -> p h d", h=NH),
            func=AF.Copy)

    # prep work interleaved into the attention stream, keyed by global group
    # index: each op lands a little before the first group that consumes it.
    sched = {}

    def at(g, fn):
        sched.setdefault(g, []).append(fn)

    t23_step(0)
    k_step(0)
    q_step(0)
    for j in range(1, 4):
        g = (2, 8, 16)[j - 1]
        at(g, (lambda j=j: (t23_step(j), k_step(j))))
    for mt in range(NMT):
        at(max(0, 2 * mt - 1), (lambda mt=mt: vt_step(mt)))
    for b in range(1, len(BLOCKS)):
        at(NGRP * (b - 1) + 24, (lambda b=b: q_step(b)))

    # ------------- attention + proj + residual, software-pipelined --------
    pairs = [(mt, h) for mt in range(NMT) for h in range(NH)]
    groups = [pairs[i:i + 2] for i in range(0, len(pairs), 2)]
    epat = CFG["epat"]
    bnd1_lag = CFG["bnd1_lag"]
    bnd2_lag = CFG["bnd2_lag"]
    neg64 = sb.tile([C, 1], F32, tag="neg64")
    nc.gpsimd.memset(neg64, float(M - MP))

    abl = CFG.get("ablate")
    pt_stale = None
    if abl == 'no_exp':
        pt_stale = ptp.tile([C, 2, 512], BF16, tag="pts")
        nc.vector.memset(pt_stale, 0.001)

    def emit_scores(grp, n0, nb, gi):
        s3 = ps.tile([C, 2, 512], F32, tag="s3", bufs=2)
        for j, (mt, h) in enumerate(grp):
            hh = 0 if abl == 'depack' else h
            nc.tensor.matmul(
                s3[:, j, 0:nb],
                k_sb[HD * hh:HD * (hh + 1), mt * C:(mt + 1) * C],
                q_sb[HD * hh:HD * (hh + 1), n0:n0 + nb],
                start=True, stop=True, tile_position=(HD * hh, 0))
        if abl == 'no_exp':
            return pt_stale
        pt = ptp.tile([C, 2, 512], BF16, tag="pt")
        if epat[gi % len(epat)] == 'D':
            nc.vector.tensor_scalar(out=pt[:, :, 0:nb].bitcast(I16),
                                    in0=s3[:, :, 0:nb],
                                    scalar1=A16, scalar2=B16,
                                    op0=ALU.mult, op1=ALU.add)
        else:
            nc.scalar.activation(out=pt[:, :, 0:nb], in_=s3[:, :, 0:nb],
                                 func=AF.Exp)
        return pt

    def emit_av(grp, banks, pt, n0, nb):
        if abl == 'no_av':
            return
        for j, (mt, h) in enumerate(grp):
            base = 64 * (h % 2)
            nc.tensor.matmul(
                banks[h][base:base + 33, 0:nb],
                vTa[:, mt, h, :],
                pt[:, j, 0:nb],
                start=(mt == 0), stop=(mt == NMT - 1),
                tile_position=(0, base), skip_group_check=True)

    # persistent denominator tile: rows {0,32,64,96} hold the 4 heads'
    # denominators (32-aligned partition bases for engine writes); all other
    # rows stay zero so the selector matmul contracts them away.
    s4_t = sb.tile([C, 512], F32R, tag="s4t")
    nc.vector.memset(s4_t.bitcast(F32), 0.0)

    zo_zero = None
    if abl == 'no_av':
        zo_zero = sb.tile([C, 512], BF16, tag="zoz")
        nc.vector.memset(zo_zero, 0.0)

    def emit_bnd1(banks, n0, nb):
        if abl == 'no_av':
            return s4_t
        for h in range(NH):
            src = banks[h][64 * (h % 2) + 32:64 * (h % 2) + 33, 0:nb]
            nc.scalar.activation(out=s4_t[32 * h:32 * h + 1, 0:nb], in_=src,
                                 func=AF.Copy)
        return s4_t

    def emit_bnd2(s4, banks, n0, nb):
        if abl == 'no_av':
            return None
        rs_ps = ps.tile([C, 512], F32, tag="s3", bufs=2)
        nc.tensor.matmul(rs_ps[:, 0:nb], ct["sel4"].bitcast(F32),
                         s4[:, 0:nb].bitcast(F32), start=True, stop=True)
        # subtract the (MP - M) padded exp(0)=1 keys, then reciprocal
        sm_sb = stg.tile([C, 512], F32, tag="sm")
        nc.scalar.activation(out=sm_sb[:, 0:nb], in_=rs_ps[:, 0:nb],
                             func=AF.Identity, bias=neg64)
        rs = stg.tile([C, 512], F32, tag="rs")
        nc.vector.reciprocal(out=rs[:, 0:nb], in_=sm_sb[:, 0:nb])
        o1 = stg.tile([C, 512], BF16, tag="o1")
        for h in range(NH):
            base = 64 * (h % 2)
            nc.vector.tensor_tensor(out=o1[HD * h:HD * (h + 1), 0:nb],
                                    in0=banks[h][base:base + 32, 0:nb],
                                    in1=rs[HD * h:HD * (h + 1), 0:nb],
                                    op=ALU.mult)
        return o1

    def emit_bnd3(o1, banks, n0, nb):
        if abl == 'no_av':
            dma.dma_start(out=out[:, n0:n0 + nb], in_=zo_zero[:, 0:nb])
            return
        z_ps = ps.tile([C, 512], F32, tag="s3", bufs=2)
        nc.tensor.matmul(z_ps[:, 0:nb], ct["wp"], o1[:, 0:nb],
                         start=True, stop=True)
        # z + pb2 on ACT (per-partition bias), then +residual on GPSIMD
        zo1 = stg.tile([C, 512], BF16, tag="zo1")
        nc.scalar.activation(out=zo1[:, 0:nb], in_=z_ps[:, 0:nb],
                             func=AF.Identity, bias=pb2)
        zo = stg.tile([C, 512], BF16, tag="zo")
        nc.gpsimd.tensor_tensor(out=zo[:, 0:nb], in0=zo1[:, 0:nb],
                                in1=x_sb[:, n0:n0 + nb], op=ALU.add)
        dma.dma_start(out=out[:, n0:n0 + nb], in_=zo[:, 0:nb])

    from collections import deque
    av_lag = CFG["av_lag"]
    pend = deque()
    prev_bnd = None
    prev_s4 = None
    prev_o1 = None
    n0 = 0
    gi = 0
    for nb in BLOCKS:
        oa = ps.tile([C, 512], F32, tag="av", bufs=4)
        ob = ps.tile([C, 512], F32, tag="av", bufs=4)
        banks = (oa, oa, ob, ob)
        for g in range(NGRP):
            pt = emit_scores(groups[g], n0, nb, gi)
            for fn in sched.pop(gi, ()):
                fn()
            if len(pend) >= av_lag:
                emit_av(*pend.popleft())
            pend.append((groups[g], banks, pt, n0, nb))
            if prev_bnd is not None:
                if g == bnd1_lag:
                    prev_s4 = emit_bnd1(*prev_bnd)
                if g == bnd2_lag:
                    prev_o1 = emit_bnd2(prev_s4, *prev_bnd)
                if g == CFG["bnd3_lag"]:
                    emit_bnd3(prev_o1, *prev_bnd)
                    prev_bnd = None
            gi += 1
        prev_bnd = (banks, n0, nb)
        n0 += nb
    while pend:
        emit_av(*pend.popleft())
    s4 = emit_bnd1(*prev_bnd)
    o1 = emit_bnd2(s4, *prev_bnd)
    emit_bnd3(o1, *prev_bnd)


def build_nc(repeats=1):
    nc = Bacc(trn_type="TRN2")
    x = nc.declare_dram_parameter("x", [C, SP], BF16, False)
    cpk = nc.declare_dram_parameter("cpk", [C, CPK_W], F32, False)
    cpb = nc.declare_dram_parameter("cpb", [C, C], BF16, False)
    gbr = nc.declare_dram_parameter("gbr", [8, C], F32R, False)
    sel4 = nc.declare_dram_parameter("sel4", [C, C], F32R, False)
    outs = [nc.declare_dram_parameter(f"out{r}" if r else "out", [C, NQ],
                                      BF16, True)
            for r in range(repeats)]
    with tile.TileContext(nc) as tc:
        with ExitStack() as cctx:
            ct = _load_consts(nc, cctx, tc, cpk, cpb, gbr, sel4)
            for r in range(repeats):
                with ExitStack() as ctx:
                    _body(nc, ctx, tc, ct, x, outs[r])
    nc.finalize()
    return nc


def get_nc(repeats=1):
    key = ("nc", repeats)
    if key not in _CACHE:
        _CACHE[key] = build_nc(repeats)
    return _CACHE[key]


def make_in_maps(x, gn_w, gn_b, qkv_w, qkv_b, proj_w, proj_b):
    x = np.asarray(x, np.float32)
    B = x.shape[0]
    scale = HD ** -0.5
    wq = np.array(qkv_w, np.float32).T.copy()                # [C, 3C]
    wq[:, 0:C] *= scale
    bq = np.array(qkv_b, np.float32).reshape(3, C).T.copy()  # [C, 3]
    bq[:, 0] *= scale
    cpk = np.zeros((C, CPK_W), np.float32)
    cpk[:, 0:3 * C] = wq
    cpk[:, 3 * C:3 * C + 3] = bq
    cpk[:, 387] = np.array(proj_b, np.float32)
    cpk[:, 388] = np.array(gn_w, np.float32)
    cpk[:, 389] = np.array(gn_b, np.float32)
    gsum = np.zeros((C, 8), np.float32)
    gsum[np.arange(C), np.arange(C) // 16] = 1.0 / 16.0
    cpk[:, 390:398] = gsum
    cpk[:, 398] = QUAKE
    cpb = np.array(proj_w, np.float32).T.astype(ml_dtypes.bfloat16)
    gbr = np.zeros((8, C), np.float32)
    gbr[np.arange(C) // 16, np.arange(C)] = 1.0
    sel4 = np.zeros((C, C), np.float32)
    sel4[32 * (np.arange(C) // HD), np.arange(C)] = 1.0
    xf = x.reshape(B, C, SP)
    in_maps = []
    for core in range(8):
        b, qd = core // 4, core % 4
        xr = np.ascontiguousarray(np.roll(xf[b], -qd * NQ, axis=1))
        in_maps.append(dict(x=xr.astype(ml_dtypes.bfloat16),
                            cpk=cpk, cpb=cpb, gbr=gbr, sel4=sel4))
    return in_maps


def assemble(results, shape):
    B = shape[0]
    out = np.empty((B, C, SP), np.float32)
    for core in range(8):
        b, qd = core // 4, core % 4
        out[b][:, qd * NQ:(qd + 1) * NQ] = np.asarray(
            results[core]["out"]).astype(np.float32)
    return out.reshape(shape)


def run(in_maps, trace=False):
    return run_bass_kernel_spmd(get_nc(), in_maps, list(range(8)), trace=trace)


def kernel(x, gn_w, gn_b, qkv_w, qkv_b, proj_w, proj_b):
    in_maps = make_in_maps(x, gn_w, gn_b, qkv_w, qkv_b, proj_w, proj_b)
    res = run(in_maps)
    return assemble(res.results, np.asarray(x).shape)
